# revision 22
# baseline (speedup 1.0000x reference)
"""RBF-kernel attention (dense_transformer) on 8 TRN2 NeuronCores.

Reference computation (B=1, S=4096, D=768, H=12, Dh=64):
    q,k,v = x@Wq, x@Wk, x@Wv               (per-head split)
    dist  = ||q_s - k_t||^2
    scores= exp(-gamma_h/8 * dist)
    out   = (scores @ v) merged @ Wo

Sharding: 8-way data parallel over query rows (512 rows/core).  Each core
computes its local K/V shard + per-head k-norms, all-gathers an augmented
K (rows: [k(64); kn_hi; kn_lo; 1; 1; 0-pad]) and V across cores, then
computes the full distance matrix for its queries with a single 80-deep
matmul per tile:
    dist[t,s] = kaug[:,t] . qaug[:,s],  qaug = [-2q; 1; 1; qn_hi; qn_lo; 0]
(contraction padded to 80: K%16 != 0 streams at half rate on the PE).
The gathers are split into five small pieces (3 head-groups of kaug, 2
column-halves of V) interleaved on the collective queue so phase B
starts right after the first piece instead of after everything.  exp
runs on the scalar engine straight out of PSUM with the per-head scale
folded in, over 1536-column groups to amortize ACT overhead.  attn@V is
computed transposed (out^T[d,s]) so no on-chip transposes are needed,
and the final Wo matmul emits the core's output slice transposed
([768, 512]); the host transposes and concatenates.  All TensorE-facing
data is bf16 (fp32 PSUM accumulation); k/q norms get a hi+lo bf16 split
so the exponent stays fp32-accurate.
"""

import numpy as np
import ml_dtypes

N_CORES = 8
S = 4096          # sequence length
D = 768           # embed dim
H = 12            # heads
DH = 64           # head dim
SL = S // N_CORES # query rows per core (512)
P = 128
KC = D // P       # contraction chunks for projections (6)
NAUG = DH + 4     # meaningful aug rows (68)
AUG = 80          # padded to mult-of-16: K%16!=0 matmuls stream at half rate
SCALE = 1.0 / np.sqrt(DH)
HPG = 2           # heads per kaug gather part
KPARTS = H // HPG # 3 kaug gather parts
DHALF = D // 2    # V gathered in two column halves

_BF16 = ml_dtypes.bfloat16


def build(neg_a):
    """Build the SPMD Bass graph. neg_a: list of 12 floats (-gamma[h]*SCALE)."""
    import concourse.bass as bass  # noqa: F401
    import concourse.mybir as mybir
    import concourse.tile as tile
    from concourse import bacc

    fb = mybir.dt.bfloat16
    f32 = mybir.dt.float32

    nc = bacc.Bacc("TRN2", target_bir_lowering=False, debug=False,
                   num_devices=N_CORES)

    xT = nc.dram_tensor("xT", [D, SL], fb, kind="ExternalInput").ap()
    wq = nc.dram_tensor("wq", [D, D], fb, kind="ExternalInput").ap()
    wk = nc.dram_tensor("wk", [D, D], fb, kind="ExternalInput").ap()
    wv = nc.dram_tensor("wv", [D, D], fb, kind="ExternalInput").ap()
    wo = nc.dram_tensor("wo", [D, D], fb, kind="ExternalInput").ap()
    outT = nc.dram_tensor("outT", [D, SL], f32, kind="ExternalOutput").ap()

    kaug_send = nc.dram_tensor("kaug_send", [H * AUG, SL], fb)
    va_send = nc.dram_tensor("va_send", [SL, DHALF], fb)
    vb_send = nc.dram_tensor("vb_send", [SL, DHALF], fb)
    kaug_gp = [nc.dram_tensor(f"kaug_g{p}", [N_CORES * HPG * AUG, SL], fb,
                              addr_space="Shared") for p in range(KPARTS)]
    va_g = nc.dram_tensor("va_g", [S, DHALF], fb, addr_space="Shared")
    vb_g = nc.dram_tensor("vb_g", [S, DHALF], fb, addr_space="Shared")
    rg = [list(range(N_CORES))]

    def kg2d(c, h):                 # gathered kaug block [AUG, SL] of core c
        p, hh = h // HPG, h % HPG
        base = (c * HPG + hh) * AUG
        return kaug_gp[p][base:base + AUG, :]

    with tile.TileContext(nc) as tc:
        with tc.tile_pool(name="persist", bufs=1) as pp:
            xT_sb = [pp.tile([P, SL], fb, name=f"xT_sb{k}") for k in range(KC)]
            wo_sb = [pp.tile([P, D], fb, name=f"wo_sb{k}") for k in range(KC)]
            qaug = [pp.tile([AUG, SL], fb, name=f"qaug{h}") for h in range(H)]
            vga_sb = [pp.tile([P, DHALF], fb, name=f"vga_sb{j}")
                      for j in range(S // P)]
            vgb_sb = [pp.tile([P, DHALF], fb, name=f"vgb_sb{j}")
                      for j in range(S // P)]
            ot_sb = [pp.tile([P, SL], fb, name=f"ot_sb{m}") for m in range(KC)]
            hsel = pp.tile([P, 2], f32, name="hsel")

            # K-path loads first: the kaug gather gates phase B.  xT on
            # the scalar queue and wk column-major on sync so the dt=0
            # projection's inputs all land within ~5us.
            for k in range(KC):
                nc.scalar.dma_start(xT_sb[k][:], xT[k * P:(k + 1) * P, :])

            # head-pair selector for partition-sum via matmul:
            # col j sums partitions j*64..j*64+63
            nc.vector.memset(hsel[:], 0.0)
            nc.vector.memset(hsel[0:DH, 0:1], 1.0)
            nc.vector.memset(hsel[DH:P, 1:2], 1.0)

            ones_sb = pp.tile([2, SL], fb, name="ones_sb")
            nc.vector.memset(ones_sb[:], 1.0)
            zeros_sb = pp.tile([AUG - NAUG, SL], fb, name="zeros_sb")
            nc.vector.memset(zeros_sb[:], 0.0)
            # [1,1,0,0,...]: K-side aug rows 66..80 in one DMA
            onz_sb = pp.tile([AUG - NAUG + 2, SL], fb, name="onz_sb")
            nc.vector.memset(onz_sb[:], 0.0)
            nc.vector.memset(onz_sb[0:2, :], 1.0)

            # ---------------- phase A: projections + aug build -------------
            with tc.tile_pool(name="psA", bufs=3, space="PSUM") as psA, \
                 tc.tile_pool(name="psN", bufs=2, space="PSUM") as psN, \
                 tc.tile_pool(name="workA", bufs=3) as wa:

                wq_sb = [wa.tile([P, D], fb, name=f"wq_sb{k}", bufs=1)
                         for k in range(KC)]
                wk_sb = [wa.tile([P, D], fb, name=f"wk_sb{k}", bufs=1)
                         for k in range(KC)]
                wv_sb = [wa.tile([P, D], fb, name=f"wv_sb{k}", bufs=1)
                         for k in range(KC)]
                for dt in range(KC):
                    for k in range(KC):
                        nc.sync.dma_start(
                            wk_sb[k][:, dt * P:(dt + 1) * P],
                            wk[k * P:(k + 1) * P, dt * P:(dt + 1) * P])
                for k in range(KC):
                    nc.sync.dma_start(wv_sb[k][:], wv[k * P:(k + 1) * P, :])

                def project_T(w_sb, dt):
                    """psum[128, SL] = (W^T x^T) rows dt*128..+128."""
                    ps = psA.tile([P, SL], f32, name=f"projT{dt}", tag="projT")
                    for k in range(KC):
                        nc.tensor.matmul(ps[:], lhsT=w_sb[k][:, dt * P:(dt + 1) * P],
                                         rhs=xT_sb[k][:], start=(k == 0),
                                         stop=(k == KC - 1))
                    return ps

                def norms(ps_bf, dt, tag):
                    """hi/lo bf16 split of per-head sum of squares.

                    Returns [34, SL] tile: rows 0:2 = hi (head pair), rows
                    32:34 = lo — 32-aligned so compute engines may write both,
                    and nhl[half::32] DMAs one head's (hi, lo) pair at once.
                    """
                    sq = wa.tile([P, SL], f32, name=f"sq_{tag}{dt}", tag="sq")
                    nc.vector.tensor_mul(sq[:], ps_bf[:], ps_bf[:])
                    nps = psN.tile([2, SL], f32, name=f"n_{tag}{dt}", tag="norm")
                    nc.tensor.matmul(nps[:], lhsT=hsel[:], rhs=sq[:],
                                     start=True, stop=True)
                    nhl = wa.tile([34, SL], fb, name=f"nhl_{tag}{dt}", tag="nhl")
                    nc.vector.tensor_copy(nhl[0:2, :], nps[:])
                    nc.vector.tensor_sub(nhl[32:34, :], nps[:], nhl[0:2, :])
                    return nhl

                # V local (natural layout, column halves); emitted after the
                # K side in program order but the scheduler overlaps it
                for tt in range(SL // P):
                    vloc = wa.tile([P, D], fb, name=f"vloc{tt}", tag="vloc")
                    for nh in range(2):
                        ps = psA.tile([P, 384], f32, name=f"vps{tt}_{nh}", tag="vps")
                        for k in range(KC):
                            nc.tensor.matmul(
                                ps[:], lhsT=xT_sb[k][:, tt * P:(tt + 1) * P],
                                rhs=wv_sb[k][:, nh * 384:(nh + 1) * 384],
                                start=(k == 0), stop=(k == KC - 1))
                        nc.vector.tensor_copy(vloc[:, nh * 384:(nh + 1) * 384], ps[:])
                    nc.scalar.dma_start(va_send[tt * P:(tt + 1) * P, :],
                                        vloc[:, :DHALF])
                    nc.scalar.dma_start(vb_send[tt * P:(tt + 1) * P, :],
                                        vloc[:, DHALF:])

                # K side: assemble kaug in DRAM piecewise (DMA has no
                # partition-alignment constraint, compute engines do);
                # gather each 4-head part as soon as it's assembled
                for dt in range(KC):
                    ps = project_T(wk_sb, dt)
                    ktb = wa.tile([P, SL], fb, name=f"ktb{dt}", tag="ktb")
                    nc.vector.tensor_copy(ktb[:], ps[:])
                    nhl = norms(ktb, dt, "k")
                    for half in range(2):
                        h = 2 * dt + half
                        r0 = h * AUG
                        nc.scalar.dma_start(kaug_send[r0:r0 + DH, :],
                                            ktb[half * DH:(half + 1) * DH, :])
                        nc.scalar.dma_start(kaug_send[r0 + DH:r0 + DH + 2, :],
                                            nhl[half:34:32, :])
                        nc.scalar.dma_start(kaug_send[r0 + DH + 2:r0 + AUG, :],
                                            onz_sb[:])
                    p = dt
                    nc.gpsimd.collective_compute(
                        "AllGather", mybir.AluOpType.bypass,
                        ins=[kaug_send[p * HPG * AUG:(p + 1) * HPG * AUG, :]],
                        outs=[kaug_gp[p][:]], replica_groups=rg)
                    if p == 0:
                        # V first half rides right after kaug part 0: heads
                        # 0-5's attn@V needs it a few us after the first EXP
                        nc.gpsimd.collective_compute(
                            "AllGather", mybir.AluOpType.bypass,
                            ins=[va_send[:]], outs=[va_g[:]],
                            replica_groups=rg)
                    if p == 2:
                        nc.gpsimd.collective_compute(
                            "AllGather", mybir.AluOpType.bypass,
                            ins=[vb_send[:]], outs=[vb_g[:]],
                            replica_groups=rg)

                # gathered V -> SBUF (resident)
                for j in range(S // P):
                    nc.gpsimd.dma_start(vga_sb[j][:], va_g[j * P:(j + 1) * P, :])
                for j in range(S // P):
                    nc.gpsimd.dma_start(vgb_sb[j][:], vb_g[j * P:(j + 1) * P, :])

                # Q side: build resident qaug tiles (overlaps the collectives)
                for k in range(KC):
                    nc.sync.dma_start(wq_sb[k][:], wq[k * P:(k + 1) * P, :])
                for dt in range(KC):
                    ps = project_T(wq_sb, dt)
                    qtb = wa.tile([P, SL], fb, name=f"qtb{dt}", tag="ktb")
                    nc.vector.tensor_copy(qtb[:], ps[:])
                    nhl = norms(qtb, dt, "q")
                    for half in range(2):
                        h = 2 * dt + half
                        qa = qaug[h]
                        nc.vector.tensor_scalar_mul(
                            qa[0:DH, :], qtb[half * DH:(half + 1) * DH, :], -2.0)
                        # rows 64-67 ([1;1;qn_hi;qn_lo]) + zero pad via DMA:
                        # partition offsets 65..67 aren't 32-aligned for
                        # compute engines
                        nc.scalar.dma_start(qa[DH:DH + 2, :], ones_sb[:])
                        nc.scalar.dma_start(qa[DH + 2:DH + 4, :],
                                            nhl[half:34:32, :])
                        nc.scalar.dma_start(qa[NAUG:AUG, :], zeros_sb[:])
                for k in range(KC):
                    nc.sync.dma_start(wo_sb[k][:], wo[k * P:(k + 1) * P, :])

            # ---------------- phase B: scores + attn@V ---------------------
            # 3 t-chunks (1536 cols) per EXP to amortize ACT's 352-cycle
            # per-instruction overhead; PSUM: 2*3 (dist) + 2*1 (o_ps) = 8
            groups = [list(range(g * 3, min(32, g * 3 + 3)))
                      for g in range((32 + 2) // 3)]

            def vslice(j, h):
                if h < 6:
                    return vga_sb[j][:, h * DH:(h + 1) * DH]
                return vgb_sb[j][:, (h - 6) * DH:(h - 5) * DH]

            with tc.tile_pool(name="psD", bufs=2, space="PSUM") as psD, \
                 tc.tile_pool(name="psO", bufs=2, space="PSUM") as psO, \
                 tc.tile_pool(name="workB", bufs=10) as wb:
                for hp in range(H // 2):
                    pair = (2 * hp, 2 * hp + 1)
                    o_pss = {}
                    kgss = {h: {} for h in pair}
                    for h in pair:
                        o_pss[h] = psO.tile([DH, SL], f32, name=f"o_ps{h}",
                                            tag="o_ps")
                    for grp in groups:
                        for h in pair:
                            kgs = kgss[h]
                            for c in sorted({j // 4 for j in grp}):
                                if c not in kgs:
                                    kg = wb.tile([AUG, SL], fb,
                                                 name=f"kg{h}_{c}", tag="kg")
                                    nc.sync.dma_start(kg[:], kg2d(c, h))
                                    kgs[c] = kg
                            w = len(grp) * SL
                            dist = psD.tile([P, 3 * SL], f32,
                                            name=f"dist{h}_{grp[0]}", tag="dist")
                            for idx, j in enumerate(grp):
                                nc.tensor.matmul(
                                    dist[:, idx * SL:(idx + 1) * SL],
                                    lhsT=kgs[j // 4][:, (j % 4) * P:(j % 4 + 1) * P],
                                    rhs=qaug[h][:], start=True, stop=True)
                            sc = wb.tile([P, 3 * SL], fb,
                                         name=f"sc{h}_{grp[0]}", tag="sc",
                                         bufs=16)
                            nc.scalar.activation(
                                sc[:, :w], dist[:, :w],
                                mybir.ActivationFunctionType.Exp,
                                scale=float(neg_a[h]))
                            for idx, j in enumerate(grp):
                                nc.tensor.matmul(
                                    o_pss[h][:], lhsT=vslice(j, h),
                                    rhs=sc[:, idx * SL:(idx + 1) * SL],
                                    start=(j == 0), stop=(j == 31))
                    for h in pair:
                        nc.vector.tensor_copy(
                            ot_sb[h // 2][(h % 2) * DH:((h % 2) + 1) * DH, :],
                            o_pss[h][:])

                # ------------- phase C: out^T = Wo^T @ O^T -----------------
                for nt in range(KC):
                    rps = psD.tile([P, 3 * SL], f32, name=f"rps{nt}", tag="dist")
                    for m in range(KC):
                        nc.tensor.matmul(rps[:, :SL],
                                         lhsT=wo_sb[m][:, nt * P:(nt + 1) * P],
                                         rhs=ot_sb[m][:], start=(m == 0),
                                         stop=(m == KC - 1))
                    rsb = wb.tile([P, SL], f32, name=f"rsb{nt}", tag="rsb")
                    nc.vector.tensor_copy(rsb[:], rps[:, :SL])
                    nc.gpsimd.dma_start(outT[nt * P:(nt + 1) * P, :], rsb[:])

    nc.compile()
    return nc


def prepare_in_maps(x, Wq, Wk, Wv, Wo):
    xT = np.ascontiguousarray(x.reshape(S, D).T)  # [768, 4096]
    wqb = Wq.astype(_BF16)
    wkb = Wk.astype(_BF16)
    wvb = Wv.astype(_BF16)
    wob = Wo.astype(_BF16)
    in_maps = []
    for c in range(N_CORES):
        in_maps.append({
            "xT": np.ascontiguousarray(xT[:, c * SL:(c + 1) * SL]).astype(_BF16),
            "wq": wqb, "wk": wkb, "wv": wvb, "wo": wob,
        })
    return in_maps


def postprocess(results):
    out = np.empty((S, D), np.float32)
    for c in range(N_CORES):
        out[c * SL:(c + 1) * SL, :] = results[c]["outT"].T
    return out.reshape(1, S, D)


_CACHE = {}


def _get_nc(gamma):
    key = tuple(np.asarray(gamma, np.float64).tolist())
    if key not in _CACHE:
        neg_a = [-float(g) * SCALE for g in gamma]
        _CACHE[key] = build(neg_a)
    return _CACHE[key]


def kernel(x, Wq, Wk, Wv, Wo, gamma):
    from concourse.bass_utils import run_bass_kernel_spmd

    x = np.asarray(x, np.float32)
    nc = _get_nc(np.asarray(gamma, np.float32))
    in_maps = prepare_in_maps(x, np.asarray(Wq, np.float32),
                              np.asarray(Wk, np.float32),
                              np.asarray(Wv, np.float32),
                              np.asarray(Wo, np.float32))
    res = run_bass_kernel_spmd(nc, in_maps, core_ids=list(range(N_CORES)))
    return postprocess(res.results)
